# revision 6
# baseline (speedup 1.0000x reference)
"""Trainium2 Bass kernel for nn_BiRNN_IFBU: 3-layer bidirectional LSTM + heads.

Strategy (v1):
  - Data-parallel over batch: 8 cores x 8 sequences, each core runs both
    directions for its sequences.
  - Mask compression: only valid timesteps are scanned (~2x fewer steps).
    Forward scans are end-padded, backward scans front-padded so the
    position flip between directions is uniform (j = L-1-k).
  - Pad steps are neutralized by an extra "flag" input feature whose weight
    row pushes gates to (i~0, f~1, o~0), freezing c and zeroing h.
  - Unit-major layout on chip: gates/h/c live as [128 partitions, ...],
    weights are the matmul stationary operand (fp16 => fast weight load).
  - Input projections (x @ Wk + b) are precomputed in bulk per layer into
    DRAM as two buffers per direction: own-half (P) and other-half (Q,
    consumed with a position flip during the scan).
  - Gate order on chip is (g, i, f, o); host permutes weight columns.
"""
import math
import numpy as np

import concourse.bass as bass
import concourse.mybir as mybir
import concourse.tile as tile
from concourse import bacc
from concourse.bass import ds, ts
from concourse.bass_utils import run_bass_kernel_spmd

F32 = mybir.dt.float32
F16 = mybir.dt.float16
AF = mybir.ActivationFunctionType
ALU = mybir.AluOpType

# problem constants
NL, U, D, B, T = 3, 512, 256, 64, 1024
PPI_OUT, BUR_OUT = 2, 1
NCORES = 8
BPC = B // NCORES          # sequences per core (both directions)
G4 = 4 * U
UC = U // 128              # unit chunks (4)
MC = G4 // 128             # gate chunks (16)
HMC = MC // 2
CRAFT = 30.0
L_MIN = 576                # keep shapes stable across seeds (cache hits)

# gate order on chip: g,i,f,o ; reference order: i,f,g,o
GATE_PERM = [2, 0, 1, 3]


def _permute_gate_cols(W):
    blocks = np.split(np.asarray(W), 4, axis=-1)
    return np.concatenate([blocks[i] for i in GATE_PERM], axis=-1)


def _craft_vec():
    v = np.zeros(G4, np.float32)
    v[1 * U:2 * U] = -CRAFT   # i
    v[2 * U:3 * U] = +CRAFT   # f
    v[3 * U:4 * U] = -CRAFT   # o
    return v


def _pack_w(W, kc_total):
    """W [K, G4] -> [128, kc_total*G4] fp16 where slice [:, kc*G4+m*128:+128]
    is the lhsT tile for (kc, m): element (p, ...) = W[kc*128+p, m*128+col]."""
    K = W.shape[0]
    Wp = np.zeros((kc_total * 128, W.shape[1]), np.float16)
    Wp[:K] = W.astype(np.float16)
    out = np.zeros((128, kc_total * W.shape[1]), np.float16)
    for kc in range(kc_total):
        out[:, kc * W.shape[1]:(kc + 1) * W.shape[1]] = Wp[kc * 128:(kc + 1) * 128]
    return out


def _shuffle_bias(b):
    """b [G4] -> [128, MC] f32 with out[p, m] = b[m*128+p]"""
    return np.ascontiguousarray(np.asarray(b, np.float32).reshape(MC, 128).T)


def build_kernel(L, nl=None, u=None, d_in=None, bpc=None):
    """Build the SPMD graph for compressed length L (L % 8 == 0)."""
    nl = NL if nl is None else nl
    u = U if u is None else u
    d_in = D if d_in is None else d_in
    bpc = BPC if bpc is None else bpc
    uc = u // 128 if u >= 128 else 1
    pu = min(u, 128)
    g4 = 4 * u
    mc = max(1, g4 // 128)
    hmc = mc // 2
    NB = L // 8
    cols = L * bpc
    blk_cols = 8 * bpc
    # n-chunk size for bulk matmuls
    g = math.gcd(NB, 8)
    NCH = blk_cols * g
    NBn = cols // NCH
    kc1 = (d_in + 1 + 127) // 128  # layer-1 K chunks (x + flag row)

    nc = bacc.Bacc(num_devices=NCORES)
    dirs = ("f", "b")

    # ---- I/O ----
    ext_in = {}

    def din(name, shape, dt):
        ext_in[name] = nc.dram_tensor(name, shape, dt, kind="ExternalInput")
        return ext_in[name]

    x_in = {dd: din(f"x{dd}", [128, kc1, cols], F16) for dd in dirs}
    fl_in = {dd: din(f"fl{dd}", [128, cols], F16) for dd in dirs}
    wk_in = {}
    wq_in = {}
    wr_in = {}
    bs_in = {}
    for l in range(1, nl + 1):
        for dd in dirs:
            kco = kc1 if l == 1 else uc + 1
            wk_in[(l, dd)] = din(f"wk{l}{dd}", [128, kco * g4], F16)
            if l > 1:
                wq_in[(l, dd)] = din(f"wq{l}{dd}", [128, uc * g4], F16)
            wr_in[(l, dd)] = din(f"wr{l}{dd}", [128, uc * g4], F16)
            bs_in[(l, dd)] = din(f"bs{l}{dd}", [128, mc], F32)
    wh_in = {dd: din(f"wh{dd}", [128, uc * 3], F16) for dd in dirs}

    out_t = {dd: nc.dram_tensor(f"out{'A' if dd == 'f' else 'B'}", [3, cols], F32,
                                kind="ExternalOutput") for dd in dirs}

    # ---- internal DRAM ----
    pown = {dd: nc.dram_tensor(f"pown{dd}", [128, NB, mc, 8, bpc], F32) for dd in dirs}
    qoth = {dd: nc.dram_tensor(f"qoth{dd}", [128, NB, mc, 8, bpc], F32) for dd in dirs}
    hbuf = {(i, dd): nc.dram_tensor(f"hbuf{i}{dd}", [128, uc, L, bpc], F16)
            for i in range(2) for dd in dirs}

    with tile.TileContext(nc) as tc:
        # ================= per-layer =================
        for l in range(1, nl + 1):
            prev = (l - 2) % 2
            cur = (l - 1) % 2
            kco = kc1 if l == 1 else uc + 1

            # ---------- bulk projection: pown (+ qoth) ----------
            with tc.tile_pool(name="pc_w", bufs=1) as wpool, \
                 tc.tile_pool(name="pc_rhs", bufs=2) as rpool, \
                 tc.tile_pool(name="pc_ps", bufs=2, space="PSUM") as pspool, \
                 tc.tile_pool(name="pc_out", bufs=3) as opool:
                for dd in dirs:
                    od = "b" if dd == "f" else "f"
                    wk_sb = wpool.tile([128, kco * g4], F16, tag="wk")
                    nc.sync.dma_start(out=wk_sb[:], in_=wk_in[(l, dd)][:])
                    bias_sb = wpool.tile([128, mc], F32, tag="bs")
                    nc.sync.dma_start(out=bias_sb[:], in_=bs_in[(l, dd)][:])
                    if l > 1:
                        wq_sb = wpool.tile([128, uc * g4], F16, tag="wq")
                        nc.sync.dma_start(out=wq_sb[:], in_=wq_in[(l, dd)][:])
                    for n in range(NBn):
                        c0 = n * NCH
                        nblk = NCH // blk_cols
                        b0 = n * nblk
                        # own rhs tiles
                        rhs = []
                        for kc in range(kco):
                            rt = rpool.tile([128, NCH], F16, tag=f"rhs{kc}")
                            if l == 1:
                                nc.sync.dma_start(out=rt[:], in_=x_in[dd][:, kc, c0:c0 + NCH])
                            elif kc < uc:
                                nc.sync.dma_start(
                                    out=rt[:],
                                    in_=hbuf[(prev, dd)][:, kc, b0 * 8:(b0 + nblk) * 8, :])
                            else:
                                nc.sync.dma_start(out=rt[:], in_=fl_in[dd][:, c0:c0 + NCH])
                            rhs.append(rt)
                        if l > 1:
                            qrhs = []
                            for kc in range(uc):
                                rt = rpool.tile([128, NCH], F16, tag=f"qrhs{kc}")
                                nc.sync.dma_start(
                                    out=rt[:],
                                    in_=hbuf[(prev, od)][:, kc, b0 * 8:(b0 + nblk) * 8, :])
                                qrhs.append(rt)
                        for m in range(mc):
                            ps = pspool.tile([128, NCH], F32, tag="ps")
                            for kc in range(kco):
                                nc.tensor.matmul(ps[:, :],
                                                 wk_sb[:, kc * g4 + m * 128:kc * g4 + m * 128 + 128],
                                                 rhs[kc][:],
                                                 start=(kc == 0), stop=(kc == kco - 1))
                            so = opool.tile([128, NCH], F32, tag="so")
                            nc.scalar.activation(so[:], ps[:], AF.Identity,
                                                 bias=bias_sb[:, m:m + 1])
                            nc.sync.dma_start(
                                out=pown[dd][:, b0:b0 + nblk, m, :, :], in_=so[:])
                            if l > 1:
                                psq = pspool.tile([128, NCH], F32, tag="psq")
                                for kc in range(uc):
                                    nc.tensor.matmul(psq[:, :],
                                                     wq_sb[:, kc * g4 + m * 128:kc * g4 + m * 128 + 128],
                                                     qrhs[kc][:],
                                                     start=(kc == 0), stop=(kc == uc - 1))
                                soq = opool.tile([128, NCH], F32, tag="soq")
                                nc.scalar.activation(soq[:], psq[:], AF.Copy)
                                nc.sync.dma_start(
                                    out=qoth[dd][:, b0:b0 + nblk, m, :, :], in_=soq[:])

            # ---------- scan ----------
            with tc.tile_pool(name="sc_w", bufs=1) as wpool, \
                 tc.tile_pool(name="sc_state", bufs=1) as stpool, \
                 tc.tile_pool(name="sc_blk", bufs=2) as bpool, \
                 tc.tile_pool(name="sc_ps", bufs=2, space="PSUM") as zpool, \
                 tc.tile_pool(name="sc_tmp", bufs=3) as tpool:
                wr_sb = {}
                c_cur = {}
                h_cur = {}
                for dd in dirs:
                    wr_sb[dd] = wpool.tile([128, uc * g4], F16, tag=f"wr{dd}", name=f"wrsb{dd}")
                    nc.sync.dma_start(out=wr_sb[dd][:], in_=wr_in[(l, dd)][:])
                    c_cur[dd] = stpool.tile([128, uc, bpc], F32, tag=f"c{dd}", name=f"ccur{dd}")
                    nc.vector.memset(c_cur[dd][:], 0.0)
                    h_cur[dd] = stpool.tile([128, uc, bpc], F16, tag=f"h{dd}", name=f"hcur{dd}")
                    nc.vector.memset(h_cur[dd][:], 0.0)

                with tc.For_i(0, NB) as ib:
                    pblk = {}
                    qblk = {}
                    hblk = {}
                    for dd in dirs:
                        pblk[dd] = bpool.tile([128, mc, 8, bpc], F32, tag=f"p{dd}", name=f"pblk{dd}")
                        nc.sync.dma_start(out=pblk[dd][:], in_=pown[dd][:, ds(ib, 1), :, :, :])
                        if l > 1:
                            qblk[dd] = bpool.tile([128, mc, 8, bpc], F32, tag=f"q{dd}", name=f"qblk{dd}")
                            nc.sync.dma_start(out=qblk[dd][:],
                                              in_=qoth[dd][:, ds(NB - 1 - ib, 1), :, :, :])
                        hblk[dd] = bpool.tile([128, uc, 8, bpc], F16, tag=f"hb{dd}", name=f"hblk{dd}")
                    for uu in range(8):
                        for dd in dirs:
                            z_lo = zpool.tile([128, hmc, bpc], F32, tag=f"zl{dd}")
                            z_hi = zpool.tile([128, hmc, bpc], F32, tag=f"zh{dd}")
                            for half, zt in ((0, z_lo), (1, z_hi)):
                                for mm in range(hmc):
                                    m = half * hmc + mm
                                    for kc in range(uc):
                                        nc.tensor.matmul(
                                            zt[:, mm, :],
                                            wr_sb[dd][:, kc * g4 + m * 128:kc * g4 + m * 128 + 128],
                                            h_cur[dd][:, kc, :],
                                            start=(kc == 0 and mm == 0),
                                            stop=(kc == uc - 1 and mm == hmc - 1))
                            zs_lo = tpool.tile([128, hmc, bpc], F32, tag=f"zsl{dd}")
                            nc.vector.tensor_tensor(zs_lo[:], z_lo[:],
                                                    pblk[dd][:, 0:hmc, uu, :], ALU.add)
                            if l > 1:
                                nc.vector.tensor_tensor(zs_lo[:], zs_lo[:],
                                                        qblk[dd][:, 0:hmc, 7 - uu, :], ALU.add)
                            g_t = tpool.tile([128, uc, bpc], F32, tag=f"g{dd}")
                            i_t = tpool.tile([128, uc, bpc], F32, tag=f"i{dd}")
                            nc.scalar.activation(g_t[:], zs_lo[:, 0:uc, :], AF.Tanh)
                            nc.scalar.activation(i_t[:], zs_lo[:, uc:2 * uc, :], AF.Sigmoid)
                            t1 = tpool.tile([128, uc, bpc], F32, tag=f"t1{dd}")
                            nc.vector.tensor_tensor(t1[:], i_t[:], g_t[:], ALU.mult)
                            zs_hi = tpool.tile([128, hmc, bpc], F32, tag=f"zsh{dd}")
                            nc.vector.tensor_tensor(zs_hi[:], z_hi[:],
                                                    pblk[dd][:, hmc:mc, uu, :], ALU.add)
                            if l > 1:
                                nc.vector.tensor_tensor(zs_hi[:], zs_hi[:],
                                                        qblk[dd][:, hmc:mc, 7 - uu, :], ALU.add)
                            fo_t = tpool.tile([128, hmc, bpc], F32, tag=f"fo{dd}")
                            nc.scalar.activation(fo_t[:], zs_hi[:], AF.Sigmoid)
                            t2 = tpool.tile([128, uc, bpc], F32, tag=f"t2{dd}")
                            nc.vector.tensor_tensor(t2[:], fo_t[:, 0:uc, :], c_cur[dd][:], ALU.mult)
                            nc.vector.tensor_tensor(c_cur[dd][:], t1[:], t2[:], ALU.add)
                            tc_t = tpool.tile([128, uc, bpc], F32, tag=f"tc{dd}")
                            nc.scalar.activation(tc_t[:], c_cur[dd][:], AF.Tanh)
                            nc.vector.tensor_tensor(h_cur[dd][:], fo_t[:, uc:, :], tc_t[:], ALU.mult)
                            nc.scalar.copy(hblk[dd][:, :, uu, :], h_cur[dd][:])
                    for dd in dirs:
                        nc.sync.dma_start(out=hbuf[(cur, dd)][:, :, ds(ib * 8, 8), :],
                                          in_=hblk[dd][:])

        # ================= heads =================
        h3 = (nl - 1) % 2
        with tc.tile_pool(name="hd_w", bufs=1) as wpool, \
             tc.tile_pool(name="hd_rhs", bufs=3) as rpool, \
             tc.tile_pool(name="hd_ps", bufs=2, space="PSUM") as pspool, \
             tc.tile_pool(name="hd_out", bufs=3) as opool:
            for dd in dirs:
                wh_sb = wpool.tile([128, uc * 3], F16, tag="wh")
                nc.sync.dma_start(out=wh_sb[:], in_=wh_in[dd][:])
                for n in range(NBn):
                    c0 = n * NCH
                    nblk = NCH // blk_cols
                    b0 = n * nblk
                    ps = pspool.tile([3, NCH], F32, tag="hps")
                    for kc in range(uc):
                        rt = rpool.tile([128, NCH], F16, tag=f"hr{kc}")
                        nc.sync.dma_start(out=rt[:],
                                          in_=hbuf[(h3, dd)][:, kc, b0 * 8:(b0 + nblk) * 8, :])
                        rr = rpool.tile([128, NCH], F16, tag=f"hrr{kc}")
                        nc.scalar.activation(rr[:], rt[:], AF.Relu)
                        nc.tensor.matmul(ps[:, :], wh_sb[:, kc * 3:kc * 3 + 3], rr[:],
                                         start=(kc == 0), stop=(kc == uc - 1))
                    so = opool.tile([3, NCH], F32, tag="hso")
                    nc.vector.tensor_copy(so[:], ps[:])
                    nc.sync.dma_start(out=out_t[dd][:, c0:c0 + NCH], in_=so[:])

    nc.compile()
    return nc


_BUILD_CACHE = {}


def _get_nc(L):
    if L not in _BUILD_CACHE:
        _BUILD_CACHE[L] = build_kernel(L)
    return _BUILD_CACHE[L]


def _prep_host(x, x_mask, lstm_params, ppi_W, ppi_b, bur_W, bur_b, L):
    """Build per-core input maps."""
    x = np.asarray(x, np.float32)
    valid = np.asarray(x_mask) != 1
    nv = valid.sum(axis=1).astype(np.int64)
    kc1 = (D + 1 + 127) // 128
    cols = L * BPC

    v_perm = _craft_vec()  # already in device gate order (g,i,f,o)

    # shared weight tensors
    shared = {}
    for l in range(1, NL + 1):
        for di, dd in enumerate(("f", "b")):
            Wk, Wr, bb = lstm_params[l - 1][di]
            Wk = _permute_gate_cols(Wk)
            Wr = _permute_gate_cols(Wr)
            bb = _permute_gate_cols(np.asarray(bb)[None, :])[0]
            if l == 1:
                aug = np.zeros((kc1 * 128, G4), np.float32)
                aug[:D] = Wk
                aug[D] = v_perm
                shared[f"wk{l}{dd}"] = _pack_w(aug, kc1)
            else:
                own = Wk[:U] if dd == "f" else Wk[U:]
                oth = Wk[U:] if dd == "f" else Wk[:U]
                aug = np.zeros(((UC + 1) * 128, G4), np.float32)
                aug[:U] = own
                aug[U] = v_perm
                shared[f"wk{l}{dd}"] = _pack_w(aug, UC + 1)
                shared[f"wq{l}{dd}"] = _pack_w(oth, UC)
            shared[f"wr{l}{dd}"] = _pack_w(Wr, UC)
            shared[f"bs{l}{dd}"] = _shuffle_bias(bb)
    Whead = np.concatenate([np.asarray(ppi_W), np.asarray(bur_W)], axis=1)  # [2U,3]
    shared["whf"] = _pack_w(Whead[:U].astype(np.float32), UC)
    shared["whb"] = _pack_w(Whead[U:].astype(np.float32), UC)

    in_maps = []
    meta = []
    for c in range(NCORES):
        sl = slice(c * BPC, (c + 1) * BPC)
        xs = x[sl]
        vs = valid[sl]
        nvs = nv[sl]
        xf = np.zeros((128, kc1, L, BPC), np.float16)
        xb = np.zeros((128, kc1, L, BPC), np.float16)
        flf = np.zeros((128, L, BPC), np.float16)
        flb = np.zeros((128, L, BPC), np.float16)
        asc = np.zeros((BPC, L), np.int64)
        for b in range(BPC):
            tsv = np.nonzero(vs[b])[0]
            n = len(tsv)
            asc[b, :n] = tsv
            xfb = xs[b, tsv]                      # [n, D]
            xbb = xfb[::-1]
            fpad = L - n
            # features 0..D-1, flag at D
            full_f = np.zeros((L, kc1 * 128), np.float32)
            full_b = np.zeros((L, kc1 * 128), np.float32)
            full_f[:n, :D] = xfb
            full_f[n:, D] = 1.0
            full_b[fpad:, :D] = xbb
            full_b[:fpad, D] = 1.0
            for kc in range(kc1):
                xf[:, kc, :, b] = full_f[:, kc * 128:(kc + 1) * 128].T.astype(np.float16)
                xb[:, kc, :, b] = full_b[:, kc * 128:(kc + 1) * 128].T.astype(np.float16)
            flf[0, :, b] = full_f[:, D].astype(np.float16)
            flb[0, :, b] = full_b[:, D].astype(np.float16)
        m = {"xf": xf.reshape(128, kc1, cols), "xb": xb.reshape(128, kc1, cols),
             "flf": flf.reshape(128, cols), "flb": flb.reshape(128, cols)}
        m.update(shared)
        in_maps.append(m)
        meta.append((vs, nvs, asc))
    return in_maps, meta


def _post_host(results, meta, ppi_b, bur_b, valid_all, L):
    head_b = np.concatenate([np.asarray(ppi_b), np.asarray(bur_b)]).astype(np.float32)
    logits = np.zeros((B, T, 3), np.float32)
    for c in range(NCORES):
        vs, nvs, asc = meta[c]
        A = np.asarray(results[c]["outA"]).reshape(3, L, BPC)
        Bc = np.asarray(results[c]["outB"]).reshape(3, L, BPC)
        for b in range(BPC):
            gb = c * BPC + b
            cum = np.cumsum(vs[b]).astype(np.int64)
            idxA = cum - 1
            a = np.where(idxA[:, None] >= 0,
                         A[:, np.clip(idxA, 0, L - 1), b].T, 0.0)
            kprime = cum - vs[b].astype(np.int64)
            jb = L - 1 - kprime
            bc = np.where(kprime[:, None] < nvs[b],
                          Bc[:, np.clip(jb, 0, L - 1), b].T, 0.0)
            logits[gb] = a + bc + head_b
    lg = logits[:, :, :2]
    e = np.exp(lg - lg.max(-1, keepdims=True))
    ppi = (e / e.sum(-1, keepdims=True)).astype(np.float32)
    buried = logits[:, :, 2:3].astype(np.float32)
    return ppi, buried


_LAST_RESULTS = {}


def kernel(x, x_mask, training, lstm_params, ppi_W, ppi_b, bur_W, bur_b,
           trace=False):
    x = np.asarray(x)
    valid = np.asarray(x_mask) != 1
    nv = valid.sum(axis=1)
    L = max(L_MIN, int(-(-int(nv.max()) // 64) * 64))
    nc = _get_nc(L)
    in_maps, meta = _prep_host(x, x_mask, lstm_params, ppi_W, ppi_b, bur_W, bur_b, L)
    res = run_bass_kernel_spmd(nc, in_maps, core_ids=list(range(NCORES)),
                               trace=trace)
    _LAST_RESULTS["res"] = res
    return _post_host(res.results, meta, ppi_b, bur_b, valid, L)


# revision 7
# speedup vs baseline: 1.1188x; 1.1188x over previous
"""Trainium2 Bass kernel for nn_BiRNN_IFBU: 3-layer bidirectional LSTM + heads.

Strategy (v2 - direction parallel):
  - 8 cores: cores 0-3 run the FORWARD direction for sequence groups 0-3
    (16 sequences each); cores 4-7 run the BACKWARD direction for the same
    groups. All fwd/bwd asymmetry lives in host-prepared per-core data;
    the SPMD graph is identical on all cores.
  - Mask compression: only valid timesteps are scanned (~1.8x fewer steps).
    Forward scans are end-padded, backward front-padded so the position
    flip between directions is uniform (j = L-1-k).
  - Pad steps neutralized by an extra "flag" input feature whose weight row
    pushes gates to (i~0, f~1, o~0) (freezes c, zeroes h). No masks needed.
  - Unit-major layout on chip; fp16 stationary weights (fast weight load).
  - Input projections precomputed in bulk per layer into DRAM, split into
    own-direction (P) and other-direction (Q) halves; Q is consumed with a
    position flip (reversed block index) during the scan.
  - Between layers, cores exchange their h via one 8-rank AllGather; each
    core reads its partner's shard at a partition-id-derived offset.
  - Gate order on chip is (g, i, f, o); host permutes weight columns.
"""
import math
import numpy as np

import concourse.bass as bass
import concourse.mybir as mybir
import concourse.tile as tile
from concourse import bacc
from concourse.bass import ds, ts
from concourse.bass_utils import run_bass_kernel_spmd

F32 = mybir.dt.float32
F16 = mybir.dt.float16
AF = mybir.ActivationFunctionType
ALU = mybir.AluOpType

# problem constants
NL, U, D, B, T = 3, 512, 256, 64, 1024
NCORES = 8
NGRP = 4                   # sequence groups
BPC = B // NGRP            # sequences per core (one direction): 16
G4 = 4 * U
UC = U // 128
MC = G4 // 128
HMC = MC // 2
CRAFT = 30.0
L_MIN = 576

# gate order on chip: g,i,f,o ; reference order: i,f,g,o
GATE_PERM = [2, 0, 1, 3]


def _permute_gate_cols(W):
    blocks = np.split(np.asarray(W), 4, axis=-1)
    return np.concatenate([blocks[i] for i in GATE_PERM], axis=-1)


def _craft_vec():
    v = np.zeros(G4, np.float32)
    v[1 * U:2 * U] = -CRAFT   # i
    v[2 * U:3 * U] = +CRAFT   # f
    v[3 * U:4 * U] = -CRAFT   # o
    return v


def _pack_w(W, kc_total):
    K = W.shape[0]
    Wp = np.zeros((kc_total * 128, W.shape[1]), np.float16)
    Wp[:K] = W.astype(np.float16)
    out = np.zeros((128, kc_total * W.shape[1]), np.float16)
    for kc in range(kc_total):
        out[:, kc * W.shape[1]:(kc + 1) * W.shape[1]] = Wp[kc * 128:(kc + 1) * 128]
    return out


def _shuffle_bias(b):
    return np.ascontiguousarray(np.asarray(b, np.float32).reshape(MC, 128).T)


def build_kernel(L, nl=None, u=None, d_in=None, bpc=None, ncores=None):
    nl = NL if nl is None else nl
    u = U if u is None else u
    d_in = D if d_in is None else d_in
    bpc = BPC if bpc is None else bpc
    ncores = NCORES if ncores is None else ncores
    uc = max(1, u // 128)
    g4 = 4 * u
    mc = max(1, g4 // 128)
    hmc = mc // 2
    NB = L // 8
    cols = L * bpc
    blk_cols = 8 * bpc
    g = math.gcd(NB, max(1, 512 // blk_cols))
    NCH = blk_cols * g
    NBn = cols // NCH
    kc1 = (d_in + 1 + 127) // 128

    nc = bacc.Bacc(num_devices=ncores)

    x_in = nc.dram_tensor("x", [128, kc1, cols], F16, kind="ExternalInput")
    fl_in = nc.dram_tensor("fl", [128, cols], F16, kind="ExternalInput")
    wk_in, wq_in, wr_in, bs_in = {}, {}, {}, {}
    for l in range(1, nl + 1):
        kco = kc1 if l == 1 else uc + 1
        wk_in[l] = nc.dram_tensor(f"wk{l}", [128, kco * g4], F16, kind="ExternalInput")
        if l > 1:
            wq_in[l] = nc.dram_tensor(f"wq{l}", [128, uc * g4], F16, kind="ExternalInput")
        wr_in[l] = nc.dram_tensor(f"wr{l}", [128, uc * g4], F16, kind="ExternalInput")
        bs_in[l] = nc.dram_tensor(f"bs{l}", [128, mc], F32, kind="ExternalInput")
    wh_in = nc.dram_tensor("wh", [128, uc * 3], F16, kind="ExternalInput")

    out_t = nc.dram_tensor("out", [3, cols], F32, kind="ExternalOutput")

    pown = nc.dram_tensor("pown", [128, NB, mc, 8, bpc], F32)
    qoth = nc.dram_tensor("qoth", [128, NB, mc, 8, bpc], F32)
    hbuf = {i: nc.dram_tensor(f"hbuf{i}", [128, uc, L, bpc], F16) for i in range(2)}
    gath = nc.dram_tensor("gath", [ncores, 128, uc, L, bpc], F16, addr_space="Shared")

    half = ncores // 2
    groups = [list(range(ncores))]

    with tile.TileContext(nc) as tc:
        pid = nc.sync.partition_id()
        s_oth = (pid + half) % ncores  # partner core's shard in gath

        for l in range(1, nl + 1):
            prev = (l - 2) % 2
            cur = (l - 1) % 2
            kco = kc1 if l == 1 else uc + 1

            # ---------- bulk projections ----------
            with tc.tile_pool(name="pc_w", bufs=1) as wpool, \
                 tc.tile_pool(name="pc_rhs", bufs=2) as rpool, \
                 tc.tile_pool(name="pc_ps", bufs=2, space="PSUM") as pspool, \
                 tc.tile_pool(name="pc_out", bufs=3) as opool:
                wk_sb = wpool.tile([128, kco * g4], F16, tag="wk")
                nc.sync.dma_start(out=wk_sb[:], in_=wk_in[l][:])
                bias_sb = wpool.tile([128, mc], F32, tag="bs")
                nc.sync.dma_start(out=bias_sb[:], in_=bs_in[l][:])
                if l > 1:
                    wq_sb = wpool.tile([128, uc * g4], F16, tag="wq")
                    nc.sync.dma_start(out=wq_sb[:], in_=wq_in[l][:])
                for n in range(NBn):
                    c0 = n * NCH
                    nblk = NCH // blk_cols
                    b0 = n * nblk
                    rhs = []
                    for kc in range(kco):
                        rt = rpool.tile([128, NCH], F16, tag=f"rhs{kc}", name=f"rt{kc}")
                        if l == 1:
                            nc.sync.dma_start(out=rt[:], in_=x_in[:, kc, c0:c0 + NCH])
                        elif kc < uc:
                            nc.sync.dma_start(
                                out=rt[:],
                                in_=hbuf[prev][:, kc, b0 * 8:(b0 + nblk) * 8, :])
                        else:
                            nc.sync.dma_start(out=rt[:], in_=fl_in[:, c0:c0 + NCH])
                        rhs.append(rt)
                    if l > 1:
                        qrhs = []
                        for kc in range(uc):
                            rt = rpool.tile([128, NCH], F16, tag=f"qrhs{kc}", name=f"qrt{kc}")
                            nc.sync.dma_start(
                                out=rt[:],
                                in_=gath[ds(s_oth, 1), :, kc, b0 * 8:(b0 + nblk) * 8, :])
                            qrhs.append(rt)
                    for m in range(mc):
                        ps = pspool.tile([128, NCH], F32, tag="ps")
                        for kc in range(kco):
                            nc.tensor.matmul(ps[:, :],
                                             wk_sb[:, kc * g4 + m * 128:kc * g4 + m * 128 + 128],
                                             rhs[kc][:],
                                             start=(kc == 0), stop=(kc == kco - 1))
                        so = opool.tile([128, NCH], F32, tag="so")
                        nc.scalar.activation(so[:], ps[:], AF.Identity,
                                             bias=bias_sb[:, m:m + 1])
                        nc.sync.dma_start(out=pown[:, b0:b0 + nblk, m, :, :], in_=so[:])
                        if l > 1:
                            psq = pspool.tile([128, NCH], F32, tag="psq")
                            for kc in range(uc):
                                nc.tensor.matmul(psq[:, :],
                                                 wq_sb[:, kc * g4 + m * 128:kc * g4 + m * 128 + 128],
                                                 qrhs[kc][:],
                                                 start=(kc == 0), stop=(kc == uc - 1))
                            soq = opool.tile([128, NCH], F32, tag="soq")
                            nc.scalar.activation(soq[:], psq[:], AF.Copy)
                            nc.sync.dma_start(out=qoth[:, b0:b0 + nblk, m, :, :], in_=soq[:])

            # ---------- scan ----------
            with tc.tile_pool(name="sc_w", bufs=1) as wpool, \
                 tc.tile_pool(name="sc_state", bufs=1) as stpool, \
                 tc.tile_pool(name="sc_blk", bufs=2) as bpool, \
                 tc.tile_pool(name="sc_ps", bufs=4, space="PSUM") as zpool, \
                 tc.tile_pool(name="sc_tmp", bufs=3) as tpool:
                wr_sb = wpool.tile([128, uc * g4], F16, tag="wr")
                nc.sync.dma_start(out=wr_sb[:], in_=wr_in[l][:])
                c_cur = stpool.tile([128, uc, bpc], F32, tag="c")
                nc.vector.memset(c_cur[:], 0.0)
                h_cur = stpool.tile([128, uc, bpc], F16, tag="h")
                nc.vector.memset(h_cur[:], 0.0)

                with tc.For_i(0, NB) as ib:
                    pblk = bpool.tile([128, mc, 8, bpc], F32, tag="p")
                    nc.sync.dma_start(out=pblk[:], in_=pown[:, ds(ib, 1), :, :, :])
                    if l > 1:
                        qblk = bpool.tile([128, mc, 8, bpc], F32, tag="q")
                        nc.sync.dma_start(out=qblk[:],
                                          in_=qoth[:, ds(NB - 1 - ib, 1), :, :, :])
                    hblk = bpool.tile([128, uc, 8, bpc], F16, tag="hb")
                    for uu in range(8):
                        z_lo = zpool.tile([128, hmc, bpc], F32, tag="zl")
                        z_hi = zpool.tile([128, hmc, bpc], F32, tag="zh")
                        for half_i, zt in ((0, z_lo), (1, z_hi)):
                            for mm in range(hmc):
                                m = half_i * hmc + mm
                                for kc in range(uc):
                                    nc.tensor.matmul(
                                        zt[:, mm, :],
                                        wr_sb[:, kc * g4 + m * 128:kc * g4 + m * 128 + 128],
                                        h_cur[:, kc, :],
                                        start=(kc == 0 and mm == 0),
                                        stop=(kc == uc - 1 and mm == hmc - 1))
                        zs_lo = tpool.tile([128, hmc, bpc], F32, tag="zsl")
                        nc.vector.tensor_tensor(zs_lo[:], z_lo[:],
                                                pblk[:, 0:hmc, uu, :], ALU.add)
                        if l > 1:
                            nc.vector.tensor_tensor(zs_lo[:], zs_lo[:],
                                                    qblk[:, 0:hmc, 7 - uu, :], ALU.add)
                        g_t = tpool.tile([128, uc, bpc], F32, tag="g")
                        i_t = tpool.tile([128, uc, bpc], F32, tag="i")
                        nc.scalar.activation(g_t[:], zs_lo[:, 0:uc, :], AF.Tanh)
                        nc.scalar.activation(i_t[:], zs_lo[:, uc:2 * uc, :], AF.Sigmoid)
                        t1 = tpool.tile([128, uc, bpc], F32, tag="t1")
                        nc.vector.tensor_tensor(t1[:], i_t[:], g_t[:], ALU.mult)
                        zs_hi = tpool.tile([128, hmc, bpc], F32, tag="zsh")
                        nc.vector.tensor_tensor(zs_hi[:], z_hi[:],
                                                pblk[:, hmc:mc, uu, :], ALU.add)
                        if l > 1:
                            nc.vector.tensor_tensor(zs_hi[:], zs_hi[:],
                                                    qblk[:, hmc:mc, 7 - uu, :], ALU.add)
                        fo_t = tpool.tile([128, hmc, bpc], F32, tag="fo")
                        nc.scalar.activation(fo_t[:], zs_hi[:], AF.Sigmoid)
                        t2 = tpool.tile([128, uc, bpc], F32, tag="t2")
                        nc.vector.tensor_tensor(t2[:], fo_t[:, 0:uc, :], c_cur[:], ALU.mult)
                        nc.vector.tensor_tensor(c_cur[:], t1[:], t2[:], ALU.add)
                        tc_t = tpool.tile([128, uc, bpc], F32, tag="tc")
                        nc.scalar.activation(tc_t[:], c_cur[:], AF.Tanh)
                        nc.vector.tensor_tensor(h_cur[:], fo_t[:, uc:, :], tc_t[:], ALU.mult)
                        nc.scalar.copy(hblk[:, :, uu, :], h_cur[:])
                    nc.sync.dma_start(out=hbuf[cur][:, :, ds(ib * 8, 8), :],
                                      in_=hblk[:])

            # ---------- exchange (not needed after last layer) ----------
            if l < nl:
                nc.gpsimd.collective_compute(
                    "AllGather", ALU.bypass,
                    replica_groups=groups,
                    ins=[hbuf[cur][:]], outs=[gath[:]],
                )

        # ---------- heads ----------
        h3 = (nl - 1) % 2
        with tc.tile_pool(name="hd_w", bufs=1) as wpool, \
             tc.tile_pool(name="hd_rhs", bufs=3) as rpool, \
             tc.tile_pool(name="hd_ps", bufs=2, space="PSUM") as pspool, \
             tc.tile_pool(name="hd_out", bufs=3) as opool:
            wh_sb = wpool.tile([128, uc * 3], F16, tag="wh")
            nc.sync.dma_start(out=wh_sb[:], in_=wh_in[:])
            for n in range(NBn):
                c0 = n * NCH
                nblk = NCH // blk_cols
                b0 = n * nblk
                ps = pspool.tile([3, NCH], F32, tag="hps")
                for kc in range(uc):
                    rt = rpool.tile([128, NCH], F16, tag=f"hr{kc}", name=f"hrt{kc}")
                    nc.sync.dma_start(out=rt[:],
                                      in_=hbuf[h3][:, kc, b0 * 8:(b0 + nblk) * 8, :])
                    rr = rpool.tile([128, NCH], F16, tag=f"hrr{kc}", name=f"hrrt{kc}")
                    nc.scalar.activation(rr[:], rt[:], AF.Relu)
                    nc.tensor.matmul(ps[:, :], wh_sb[:, kc * 3:kc * 3 + 3], rr[:],
                                     start=(kc == 0), stop=(kc == uc - 1))
                so = opool.tile([3, NCH], F32, tag="hso")
                nc.vector.tensor_copy(so[:], ps[:])
                nc.sync.dma_start(out=out_t[:, c0:c0 + NCH], in_=so[:])

    nc.compile()
    return nc


_BUILD_CACHE = {}


def _get_nc(L):
    if L not in _BUILD_CACHE:
        _BUILD_CACHE[L] = build_kernel(L)
    return _BUILD_CACHE[L]


def _prep_host(x, x_mask, lstm_params, ppi_W, ppi_b, bur_W, bur_b, L):
    x = np.asarray(x, np.float32)
    valid = np.asarray(x_mask) != 1
    nv = valid.sum(axis=1).astype(np.int64)
    kc1 = (D + 1 + 127) // 128
    cols = L * BPC

    v_perm = _craft_vec()  # already in device gate order (g,i,f,o)

    # per-direction weight dicts
    shared = {"f": {}, "b": {}}
    for l in range(1, NL + 1):
        for di, dd in enumerate(("f", "b")):
            Wk, Wr, bb = lstm_params[l - 1][di]
            Wk = _permute_gate_cols(Wk)
            Wr = _permute_gate_cols(Wr)
            bb = _permute_gate_cols(np.asarray(bb)[None, :])[0]
            if l == 1:
                aug = np.zeros((kc1 * 128, G4), np.float32)
                aug[:D] = Wk
                aug[D] = v_perm
                shared[dd][f"wk{l}"] = _pack_w(aug, kc1)
            else:
                own = Wk[:U] if dd == "f" else Wk[U:]
                oth = Wk[U:] if dd == "f" else Wk[:U]
                aug = np.zeros(((UC + 1) * 128, G4), np.float32)
                aug[:U] = own
                aug[U] = v_perm
                shared[dd][f"wk{l}"] = _pack_w(aug, UC + 1)
                shared[dd][f"wq{l}"] = _pack_w(oth, UC)
            shared[dd][f"wr{l}"] = _pack_w(Wr, UC)
            shared[dd][f"bs{l}"] = _shuffle_bias(bb)
    Whead = np.concatenate([np.asarray(ppi_W), np.asarray(bur_W)], axis=1)
    shared["f"]["wh"] = _pack_w(Whead[:U].astype(np.float32), UC)
    shared["b"]["wh"] = _pack_w(Whead[U:].astype(np.float32), UC)

    in_maps = []
    meta = []
    for c in range(NCORES):
        dd = "f" if c < NCORES // 2 else "b"
        grp = c % NGRP
        sl = slice(grp * BPC, (grp + 1) * BPC)
        xs = x[sl]
        vs = valid[sl]
        nvs = nv[sl]
        xd = np.zeros((128, kc1, L, BPC), np.float16)
        fld = np.zeros((128, L, BPC), np.float16)
        for b in range(BPC):
            tsv = np.nonzero(vs[b])[0]
            n = len(tsv)
            full = np.zeros((L, kc1 * 128), np.float32)
            if dd == "f":
                full[:n, :D] = xs[b, tsv]
                full[n:, D] = 1.0
            else:
                fpad = L - n
                full[fpad:, :D] = xs[b, tsv[::-1]]
                full[:fpad, D] = 1.0
            for kc in range(kc1):
                xd[:, kc, :, b] = full[:, kc * 128:(kc + 1) * 128].T.astype(np.float16)
            fld[0, :, b] = full[:, D].astype(np.float16)
        m = {"x": xd.reshape(128, kc1, cols), "fl": fld.reshape(128, cols)}
        m.update(shared[dd])
        in_maps.append(m)
        meta.append((dd, vs, nvs))
    return in_maps, meta


def _post_host(results, meta, ppi_b, bur_b, L):
    head_b = np.concatenate([np.asarray(ppi_b), np.asarray(bur_b)]).astype(np.float32)
    logits = np.zeros((B, T, 3), np.float32)
    for grp in range(NGRP):
        _, vs, nvs = meta[grp]
        A = np.asarray(results[grp]["out"]).reshape(3, L, BPC)
        Bc = np.asarray(results[grp + NGRP]["out"]).reshape(3, L, BPC)
        for b in range(BPC):
            gb = grp * BPC + b
            cum = np.cumsum(vs[b]).astype(np.int64)
            idxA = cum - 1
            a = np.where(idxA[:, None] >= 0,
                         A[:, np.clip(idxA, 0, L - 1), b].T, 0.0)
            kprime = cum - vs[b].astype(np.int64)
            jb = L - 1 - kprime
            bc = np.where(kprime[:, None] < nvs[b],
                          Bc[:, np.clip(jb, 0, L - 1), b].T, 0.0)
            logits[gb] = a + bc + head_b
    lg = logits[:, :, :2]
    e = np.exp(lg - lg.max(-1, keepdims=True))
    ppi = (e / e.sum(-1, keepdims=True)).astype(np.float32)
    buried = logits[:, :, 2:3].astype(np.float32)
    return ppi, buried


_LAST_RESULTS = {}


def kernel(x, x_mask, training, lstm_params, ppi_W, ppi_b, bur_W, bur_b,
           trace=False):
    x = np.asarray(x)
    valid = np.asarray(x_mask) != 1
    nv = valid.sum(axis=1)
    L = max(L_MIN, int(-(-int(nv.max()) // 64) * 64))
    nc = _get_nc(L)
    in_maps, meta = _prep_host(x, x_mask, lstm_params, ppi_W, ppi_b, bur_W, bur_b, L)
    res = run_bass_kernel_spmd(nc, in_maps, core_ids=list(range(NCORES)),
                               trace=trace)
    _LAST_RESULTS["res"] = res
    return _post_host(res.results, meta, ppi_b, bur_b, L)


# revision 8
# speedup vs baseline: 1.1616x; 1.0382x over previous
"""Trainium2 Bass kernel for nn_BiRNN_IFBU: 3-layer bidirectional LSTM + heads.

Strategy (v2 - direction parallel):
  - 8 cores: cores 0-3 run the FORWARD direction for sequence groups 0-3
    (16 sequences each); cores 4-7 run the BACKWARD direction for the same
    groups. All fwd/bwd asymmetry lives in host-prepared per-core data;
    the SPMD graph is identical on all cores.
  - Mask compression: only valid timesteps are scanned (~1.8x fewer steps).
    Forward scans are end-padded, backward front-padded so the position
    flip between directions is uniform (j = L-1-k).
  - Pad steps neutralized by an extra "flag" input feature whose weight row
    pushes gates to (i~0, f~1, o~0) (freezes c, zeroes h). No masks needed.
  - Unit-major layout on chip; fp16 stationary weights (fast weight load).
  - Input projections precomputed in bulk per layer into DRAM, split into
    own-direction (P) and other-direction (Q) halves; Q is consumed with a
    position flip (reversed block index) during the scan.
  - Between layers, cores exchange their h via one 8-rank AllGather; each
    core reads its partner's shard at a partition-id-derived offset.
  - Gate order on chip is (g, i, f, o); host permutes weight columns.
"""
import math
import numpy as np

import concourse.bass as bass
import concourse.mybir as mybir
import concourse.tile as tile
from concourse import bacc
from concourse.bass import ds, ts
from concourse.bass_utils import run_bass_kernel_spmd

F32 = mybir.dt.float32
F16 = mybir.dt.float16
AF = mybir.ActivationFunctionType
ALU = mybir.AluOpType

# problem constants
NL, U, D, B, T = 3, 512, 256, 64, 1024
NCORES = 8
NGRP = 4                   # sequence groups
BPC = B // NGRP            # sequences per core (one direction): 16
G4 = 4 * U
UC = U // 128
MC = G4 // 128
HMC = MC // 2
CRAFT = 30.0
L_MIN = 576

# gate order on chip: g,i,f,o ; reference order: i,f,g,o
GATE_PERM = [2, 0, 1, 3]


def _permute_gate_cols(W):
    blocks = np.split(np.asarray(W), 4, axis=-1)
    return np.concatenate([blocks[i] for i in GATE_PERM], axis=-1)


def _craft_vec():
    v = np.zeros(G4, np.float32)
    v[1 * U:2 * U] = -CRAFT   # i
    v[2 * U:3 * U] = +CRAFT   # f
    v[3 * U:4 * U] = -CRAFT   # o
    return v


def _pack_w(W, kc_total):
    K = W.shape[0]
    Wp = np.zeros((kc_total * 128, W.shape[1]), np.float16)
    Wp[:K] = W.astype(np.float16)
    out = np.zeros((128, kc_total * W.shape[1]), np.float16)
    for kc in range(kc_total):
        out[:, kc * W.shape[1]:(kc + 1) * W.shape[1]] = Wp[kc * 128:(kc + 1) * 128]
    return out


def _shuffle_bias(b):
    return np.ascontiguousarray(np.asarray(b, np.float32).reshape(MC, 128).T)


def build_kernel(L, nl=None, u=None, d_in=None, bpc=None, ncores=None):
    nl = NL if nl is None else nl
    u = U if u is None else u
    d_in = D if d_in is None else d_in
    bpc = BPC if bpc is None else bpc
    ncores = NCORES if ncores is None else ncores
    uc = max(1, u // 128)
    g4 = 4 * u
    mc = max(1, g4 // 128)
    hmc = mc // 2
    NB = L // 8
    cols = L * bpc
    blk_cols = 8 * bpc
    g = math.gcd(NB, max(1, 512 // blk_cols))
    NCH = blk_cols * g
    NBn = cols // NCH
    kc1 = (d_in + 1 + 127) // 128

    nc = bacc.Bacc(num_devices=ncores)

    x_in = nc.dram_tensor("x", [128, kc1, cols], F16, kind="ExternalInput")
    fl_in = nc.dram_tensor("fl", [128, cols], F16, kind="ExternalInput")
    wk_in, wq_in, wr_in, bs_in = {}, {}, {}, {}
    for l in range(1, nl + 1):
        kco = kc1 if l == 1 else uc + 1
        wk_in[l] = nc.dram_tensor(f"wk{l}", [128, kco * g4], F16, kind="ExternalInput")
        if l > 1:
            wq_in[l] = nc.dram_tensor(f"wq{l}", [128, uc * g4], F16, kind="ExternalInput")
        wr_in[l] = nc.dram_tensor(f"wr{l}", [128, uc * g4], F16, kind="ExternalInput")
        bs_in[l] = nc.dram_tensor(f"bs{l}", [128, mc], F32, kind="ExternalInput")
    wh_in = nc.dram_tensor("wh", [128, uc * 3], F16, kind="ExternalInput")

    out_t = nc.dram_tensor("out", [3, cols], F32, kind="ExternalOutput")

    pown = nc.dram_tensor("pown", [128, NB, mc, 8, bpc], F32)
    qoth = nc.dram_tensor("qoth", [128, NB, mc, 8, bpc], F32)
    hbuf = {i: nc.dram_tensor(f"hbuf{i}", [128, uc, L, bpc], F16) for i in range(2)}
    gath = nc.dram_tensor("gath", [ncores, 128, uc, L, bpc], F16, addr_space="Shared")

    half = ncores // 2
    groups = [list(range(ncores))]

    with tile.TileContext(nc) as tc:
        pid = nc.sync.partition_id()
        s_oth = (pid + half) % ncores  # partner core's shard in gath

        for l in range(1, nl + 1):
            prev = (l - 2) % 2
            cur = (l - 1) % 2
            kco = kc1 if l == 1 else uc + 1

            # ---------- bulk projections ----------
            with tc.tile_pool(name="pc_w", bufs=1) as wpool, \
                 tc.tile_pool(name="pc_rhs", bufs=2) as rpool, \
                 tc.tile_pool(name="pc_ps", bufs=2, space="PSUM") as pspool, \
                 tc.tile_pool(name="pc_out", bufs=3) as opool:
                wk_sb = wpool.tile([128, kco * g4], F16, tag="wk")
                nc.sync.dma_start(out=wk_sb[:], in_=wk_in[l][:])
                bias_sb = wpool.tile([128, mc], F32, tag="bs")
                nc.sync.dma_start(out=bias_sb[:], in_=bs_in[l][:])
                if l > 1:
                    wq_sb = wpool.tile([128, uc * g4], F16, tag="wq")
                    nc.sync.dma_start(out=wq_sb[:], in_=wq_in[l][:])
                for n in range(NBn):
                    c0 = n * NCH
                    nblk = NCH // blk_cols
                    b0 = n * nblk
                    rhs = []
                    for kc in range(kco):
                        rt = rpool.tile([128, NCH], F16, tag=f"rhs{kc}", name=f"rt{kc}")
                        if l == 1:
                            nc.sync.dma_start(out=rt[:], in_=x_in[:, kc, c0:c0 + NCH])
                        elif kc < uc:
                            nc.sync.dma_start(
                                out=rt[:],
                                in_=hbuf[prev][:, kc, b0 * 8:(b0 + nblk) * 8, :])
                        else:
                            nc.sync.dma_start(out=rt[:], in_=fl_in[:, c0:c0 + NCH])
                        rhs.append(rt)
                    if l > 1:
                        qrhs = []
                        for kc in range(uc):
                            rt = rpool.tile([128, NCH], F16, tag=f"qrhs{kc}", name=f"qrt{kc}")
                            nc.sync.dma_start(
                                out=rt[:],
                                in_=gath[ds(s_oth, 1), :, kc, b0 * 8:(b0 + nblk) * 8, :])
                            qrhs.append(rt)
                    for m in range(mc):
                        ps = pspool.tile([128, NCH], F32, tag="ps")
                        for kc in range(kco):
                            nc.tensor.matmul(ps[:, :],
                                             wk_sb[:, kc * g4 + m * 128:kc * g4 + m * 128 + 128],
                                             rhs[kc][:],
                                             start=(kc == 0), stop=(kc == kco - 1))
                        so = opool.tile([128, NCH], F32, tag="so")
                        nc.scalar.activation(so[:], ps[:], AF.Identity,
                                             bias=bias_sb[:, m:m + 1])
                        nc.sync.dma_start(out=pown[:, b0:b0 + nblk, m, :, :], in_=so[:])
                        if l > 1:
                            psq = pspool.tile([128, NCH], F32, tag="psq")
                            for kc in range(uc):
                                nc.tensor.matmul(psq[:, :],
                                                 wq_sb[:, kc * g4 + m * 128:kc * g4 + m * 128 + 128],
                                                 qrhs[kc][:],
                                                 start=(kc == 0), stop=(kc == uc - 1))
                            soq = opool.tile([128, NCH], F32, tag="soq")
                            nc.scalar.activation(soq[:], psq[:], AF.Copy)
                            nc.sync.dma_start(out=qoth[:, b0:b0 + nblk, m, :, :], in_=soq[:])

            # ---------- scan ----------
            with tc.tile_pool(name="sc_w", bufs=1) as wpool, \
                 tc.tile_pool(name="sc_state", bufs=1) as stpool, \
                 tc.tile_pool(name="sc_blk", bufs=2) as bpool, \
                 tc.tile_pool(name="sc_ps", bufs=4, space="PSUM") as zpool, \
                 tc.tile_pool(name="sc_tmp", bufs=3) as tpool:
                wr_sb = wpool.tile([128, uc * g4], F16, tag="wr")
                nc.sync.dma_start(out=wr_sb[:], in_=wr_in[l][:])
                c_cur = stpool.tile([128, uc, bpc], F32, tag="c")
                nc.vector.memset(c_cur[:], 0.0)
                h_cur = stpool.tile([128, uc, bpc], F16, tag="h")
                nc.vector.memset(h_cur[:], 0.0)

                with tc.For_i(0, NB, hint_engines=(mybir.EngineType.PE,)) as ib:
                    pblk = bpool.tile([128, mc, 8, bpc], F32, tag="p")
                    nc.sync.dma_start(out=pblk[:], in_=pown[:, ds(ib, 1), :, :, :])
                    if l > 1:
                        qblk = bpool.tile([128, mc, 8, bpc], F32, tag="q")
                        nc.sync.dma_start(out=qblk[:],
                                          in_=qoth[:, ds(NB - 1 - ib, 1), :, :, :])
                        # combine P[u] + Q[7-u] off the critical per-step path
                        zq = bpool.tile([128, mc, 8, bpc], F32, tag="zq")
                        for uu in range(8):
                            nc.vector.tensor_tensor(zq[:, :, uu, :], pblk[:, :, uu, :],
                                                    qblk[:, :, 7 - uu, :], ALU.add)
                    else:
                        zq = pblk
                    hblk = bpool.tile([128, uc, 8, bpc], F16, tag="hb")
                    for uu in range(8):
                        z_lo = zpool.tile([128, hmc, bpc], F32, tag="zl")
                        z_hi = zpool.tile([128, hmc, bpc], F32, tag="zh")
                        for half_i, zt in ((0, z_lo), (1, z_hi)):
                            for mm in range(hmc):
                                m = half_i * hmc + mm
                                for kc in range(uc):
                                    nc.tensor.matmul(
                                        zt[:, mm, :],
                                        wr_sb[:, kc * g4 + m * 128:kc * g4 + m * 128 + 128],
                                        h_cur[:, kc, :],
                                        start=(kc == 0 and mm == 0),
                                        stop=(kc == uc - 1 and mm == hmc - 1))
                        zs_lo = tpool.tile([128, hmc, bpc], F32, tag="zsl")
                        nc.vector.tensor_tensor(zs_lo[:], z_lo[:],
                                                zq[:, 0:hmc, uu, :], ALU.add)
                        g_t = tpool.tile([128, uc, bpc], F32, tag="g")
                        i_t = tpool.tile([128, uc, bpc], F32, tag="i")
                        nc.scalar.activation(g_t[:], zs_lo[:, 0:uc, :], AF.Tanh)
                        nc.scalar.activation(i_t[:], zs_lo[:, uc:2 * uc, :], AF.Sigmoid)
                        t1 = tpool.tile([128, uc, bpc], F32, tag="t1")
                        nc.vector.tensor_tensor(t1[:], i_t[:], g_t[:], ALU.mult)
                        zs_hi = tpool.tile([128, hmc, bpc], F32, tag="zsh")
                        nc.vector.tensor_tensor(zs_hi[:], z_hi[:],
                                                zq[:, hmc:mc, uu, :], ALU.add)
                        f_t = tpool.tile([128, uc, bpc], F32, tag="f")
                        nc.scalar.activation(f_t[:], zs_hi[:, 0:uc, :], AF.Sigmoid)
                        t2 = tpool.tile([128, uc, bpc], F32, tag="t2")
                        nc.vector.tensor_tensor(t2[:], f_t[:], c_cur[:], ALU.mult)
                        o_t = tpool.tile([128, uc, bpc], F32, tag="o")
                        nc.scalar.activation(o_t[:], zs_hi[:, uc:, :], AF.Sigmoid)
                        nc.vector.tensor_tensor(c_cur[:], t1[:], t2[:], ALU.add)
                        tc_t = tpool.tile([128, uc, bpc], F32, tag="tc")
                        nc.scalar.activation(tc_t[:], c_cur[:], AF.Tanh)
                        nc.vector.tensor_tensor(h_cur[:], o_t[:], tc_t[:], ALU.mult)
                        nc.scalar.copy(hblk[:, :, uu, :], h_cur[:])
                    nc.sync.dma_start(out=hbuf[cur][:, :, ds(ib * 8, 8), :],
                                      in_=hblk[:])

            # ---------- exchange (not needed after last layer) ----------
            if l < nl:
                nc.gpsimd.collective_compute(
                    "AllGather", ALU.bypass,
                    replica_groups=groups,
                    ins=[hbuf[cur][:]], outs=[gath[:]],
                )

        # ---------- heads ----------
        h3 = (nl - 1) % 2
        with tc.tile_pool(name="hd_w", bufs=1) as wpool, \
             tc.tile_pool(name="hd_rhs", bufs=3) as rpool, \
             tc.tile_pool(name="hd_ps", bufs=2, space="PSUM") as pspool, \
             tc.tile_pool(name="hd_out", bufs=3) as opool:
            wh_sb = wpool.tile([128, uc * 3], F16, tag="wh")
            nc.sync.dma_start(out=wh_sb[:], in_=wh_in[:])
            for n in range(NBn):
                c0 = n * NCH
                nblk = NCH // blk_cols
                b0 = n * nblk
                ps = pspool.tile([3, NCH], F32, tag="hps")
                for kc in range(uc):
                    rt = rpool.tile([128, NCH], F16, tag=f"hr{kc}", name=f"hrt{kc}")
                    nc.sync.dma_start(out=rt[:],
                                      in_=hbuf[h3][:, kc, b0 * 8:(b0 + nblk) * 8, :])
                    rr = rpool.tile([128, NCH], F16, tag=f"hrr{kc}", name=f"hrrt{kc}")
                    nc.scalar.activation(rr[:], rt[:], AF.Relu)
                    nc.tensor.matmul(ps[:, :], wh_sb[:, kc * 3:kc * 3 + 3], rr[:],
                                     start=(kc == 0), stop=(kc == uc - 1))
                so = opool.tile([3, NCH], F32, tag="hso")
                nc.vector.tensor_copy(so[:], ps[:])
                nc.sync.dma_start(out=out_t[:, c0:c0 + NCH], in_=so[:])

    nc.compile()
    return nc


_BUILD_CACHE = {}


def _get_nc(L):
    if L not in _BUILD_CACHE:
        _BUILD_CACHE[L] = build_kernel(L)
    return _BUILD_CACHE[L]


def _prep_host(x, x_mask, lstm_params, ppi_W, ppi_b, bur_W, bur_b, L):
    x = np.asarray(x, np.float32)
    valid = np.asarray(x_mask) != 1
    nv = valid.sum(axis=1).astype(np.int64)
    kc1 = (D + 1 + 127) // 128
    cols = L * BPC

    v_perm = _craft_vec()  # already in device gate order (g,i,f,o)

    # per-direction weight dicts
    shared = {"f": {}, "b": {}}
    for l in range(1, NL + 1):
        for di, dd in enumerate(("f", "b")):
            Wk, Wr, bb = lstm_params[l - 1][di]
            Wk = _permute_gate_cols(Wk)
            Wr = _permute_gate_cols(Wr)
            bb = _permute_gate_cols(np.asarray(bb)[None, :])[0]
            if l == 1:
                aug = np.zeros((kc1 * 128, G4), np.float32)
                aug[:D] = Wk
                aug[D] = v_perm
                shared[dd][f"wk{l}"] = _pack_w(aug, kc1)
            else:
                own = Wk[:U] if dd == "f" else Wk[U:]
                oth = Wk[U:] if dd == "f" else Wk[:U]
                aug = np.zeros(((UC + 1) * 128, G4), np.float32)
                aug[:U] = own
                aug[U] = v_perm
                shared[dd][f"wk{l}"] = _pack_w(aug, UC + 1)
                shared[dd][f"wq{l}"] = _pack_w(oth, UC)
            shared[dd][f"wr{l}"] = _pack_w(Wr, UC)
            shared[dd][f"bs{l}"] = _shuffle_bias(bb)
    Whead = np.concatenate([np.asarray(ppi_W), np.asarray(bur_W)], axis=1)
    shared["f"]["wh"] = _pack_w(Whead[:U].astype(np.float32), UC)
    shared["b"]["wh"] = _pack_w(Whead[U:].astype(np.float32), UC)

    in_maps = []
    meta = []
    for c in range(NCORES):
        dd = "f" if c < NCORES // 2 else "b"
        grp = c % NGRP
        sl = slice(grp * BPC, (grp + 1) * BPC)
        xs = x[sl]
        vs = valid[sl]
        nvs = nv[sl]
        xd = np.zeros((128, kc1, L, BPC), np.float16)
        fld = np.zeros((128, L, BPC), np.float16)
        for b in range(BPC):
            tsv = np.nonzero(vs[b])[0]
            n = len(tsv)
            full = np.zeros((L, kc1 * 128), np.float32)
            if dd == "f":
                full[:n, :D] = xs[b, tsv]
                full[n:, D] = 1.0
            else:
                fpad = L - n
                full[fpad:, :D] = xs[b, tsv[::-1]]
                full[:fpad, D] = 1.0
            for kc in range(kc1):
                xd[:, kc, :, b] = full[:, kc * 128:(kc + 1) * 128].T.astype(np.float16)
            fld[0, :, b] = full[:, D].astype(np.float16)
        m = {"x": xd.reshape(128, kc1, cols), "fl": fld.reshape(128, cols)}
        m.update(shared[dd])
        in_maps.append(m)
        meta.append((dd, vs, nvs))
    return in_maps, meta


def _post_host(results, meta, ppi_b, bur_b, L):
    head_b = np.concatenate([np.asarray(ppi_b), np.asarray(bur_b)]).astype(np.float32)
    logits = np.zeros((B, T, 3), np.float32)
    for grp in range(NGRP):
        _, vs, nvs = meta[grp]
        A = np.asarray(results[grp]["out"]).reshape(3, L, BPC)
        Bc = np.asarray(results[grp + NGRP]["out"]).reshape(3, L, BPC)
        for b in range(BPC):
            gb = grp * BPC + b
            cum = np.cumsum(vs[b]).astype(np.int64)
            idxA = cum - 1
            a = np.where(idxA[:, None] >= 0,
                         A[:, np.clip(idxA, 0, L - 1), b].T, 0.0)
            kprime = cum - vs[b].astype(np.int64)
            jb = L - 1 - kprime
            bc = np.where(kprime[:, None] < nvs[b],
                          Bc[:, np.clip(jb, 0, L - 1), b].T, 0.0)
            logits[gb] = a + bc + head_b
    lg = logits[:, :, :2]
    e = np.exp(lg - lg.max(-1, keepdims=True))
    ppi = (e / e.sum(-1, keepdims=True)).astype(np.float32)
    buried = logits[:, :, 2:3].astype(np.float32)
    return ppi, buried


_LAST_RESULTS = {}


def kernel(x, x_mask, training, lstm_params, ppi_W, ppi_b, bur_W, bur_b,
           trace=False):
    x = np.asarray(x)
    valid = np.asarray(x_mask) != 1
    nv = valid.sum(axis=1)
    L = max(L_MIN, int(-(-int(nv.max()) // 64) * 64))
    nc = _get_nc(L)
    in_maps, meta = _prep_host(x, x_mask, lstm_params, ppi_W, ppi_b, bur_W, bur_b, L)
    res = run_bass_kernel_spmd(nc, in_maps, core_ids=list(range(NCORES)),
                               trace=trace)
    _LAST_RESULTS["res"] = res
    return _post_host(res.results, meta, ppi_b, bur_b, L)
